# revision 29
# baseline (speedup 1.0000x reference)
"""ArcFace loss kernel for 8 TRN2 NeuronCores (vocab/tensor-parallel).

reference:
    xn = normalize(x)               # [B, D]
    wn = normalize(weight)          # [C, D]
    logits = 64 * xn @ wn.T         # [B, C]
    loss = mean(CE(logits, label))

Strategy: shard classes C=100000 over 8 cores (12500 each, exact - no
padding). Host prepares normalized, transposed fp8(e4m3) operands scaled
by G=8 (so device cosines are 64*cos and the exp scale is 1); each core
computes its logit shard with TensorE fp8 DoubleRow matmuls (K=256 per
op) into fp32 PSUM and a fused exp+row-sum on ScalarE with a fixed shift
(logsumexp(l) = SHIFT + log(sum(exp(l - SHIFT))), exact since l <= 64).

v2 pipeline notes (from trace analysis of the 77.8us baseline):
  - ScalarE EXP stream is the pacer (~55us busy incl per-op overhead);
    critical path = preamble(5.7us fixed) + time-to-first-EXP + packed
    EXP stream + drain.
  - The 212-class tail chunk is processed as the FIRST group: it needs
    only 106KB of weights, so its 4 small EXPs start ~level with the
    first weight DMAs landing and fill the window while the first full
    1MB group streams in.
  - Warmup matmuls use a K=128 bf16 stationary (not K=1): the TRN2 PE
    p-state governor needs real array utilization to ramp 0.65->2.4GHz,
    and any PE idle gap during the ramp resets it (427ns/MM at MID).
  - DMA descriptor generation costs ~640ns per dma_start on the issuing
    engine; issue is spread over sync/vector/scalar/gpsimd so the first
    group's pieces are all in flight ~2 waves after the preamble.
  - Each core returns raw per-(row, bblock, group) partials [128, 28];
    the host sums group columns, so no on-device reduce/extra sync.
"""

import math
import numpy as np

import concourse.mybir as mybir
import concourse.tile as tile
from concourse import bacc
from concourse.bass_utils import run_bass_kernel_spmd

# Problem constants (hardcoded per harness contract).
B = 512
D = 512
C = 100000
S = 64.0
SHIFT = 20.0  # logsumexp shift; keeps Z ~1e-2 (HW Ln saturates below ~1e-19)
EPS = 1e-12
G = 8.0      # fp8 pre-scale on both operands: device cos' = G^2 * cos
NCORES = 8
CS = C // NCORES        # classes per core = 12500 (exact, no padding)
CHUNK = 512             # matmul moving free dim = one full PSUM bank
GROUP = 4               # psum banks per exp/accumulate group
PB = 128                # partitions
KSUB = D // PB          # 4 contraction subtiles of 128
BBLK = B // PB          # 4 batch blocks
TAIL = CS - 24 * CHUNK  # 212: ragged tail chunk, processed first
N_WARM = 6             # fat PE warm-up matmuls (K=128) to ramp the clock

F32 = mybir.dt.float32
BF16 = mybir.dt.bfloat16
FP8 = mybir.dt.float8e4
NP_FP8 = mybir.dt.np(FP8)
EXP_SCALE = S / (G * G)  # = 1.0

# groups: [tail 212] + two 2-chunk groups (their 512KB lands early and
# fills the EXP stream while the first 1MB group is still in flight) +
# five full 4-chunk groups. (col0, ncols) per group.
GROUPS = [(24 * CHUNK, TAIL), (0, 2 * CHUNK), (2 * CHUNK, 2 * CHUNK)] + [
    (4 * CHUNK + g * GROUP * CHUNK, GROUP * CHUNK) for g in range(5)
]
NGROUPS = len(GROUPS)  # 8


def build_nc(ncores: int = NCORES):
    """Build the SPMD Bass graph."""
    nc = bacc.Bacc(
        "TRN2",
        target_bir_lowering=False,
        debug=False,
        num_devices=ncores,
    )

    # Host-packed layouts: per-partition contiguous so each group loads in
    # 1-2 dma_starts with multi-KB descriptors (descriptor GENERATION on
    # the issuing engine, ~0.62us per dma_start, is the startup
    # bottleneck -- the fewer dma_starts, the earlier everything lands).
    #   wnt: [128p, 848 + 6*8192] -- tail block [ks][212] then per full
    #        group [q(4)][ks(4)][512] (matmul rhs = [p, q, 2ks, 512]).
    #   xnt: [128p, KSUB*B] -- [ks][b].
    wnt_ext = nc.dram_tensor("wnt", [PB, KSUB * CS], FP8, kind="ExternalInput")
    xnt_ext = nc.dram_tensor("xnt", [PB, KSUB * B], FP8, kind="ExternalInput")
    zp_ext = nc.dram_tensor("zp", [PB, BBLK * NGROUPS], F32, kind="ExternalOutput")

    with tile.TileContext(nc) as tc:
        with (
            tc.tile_pool(name="const", bufs=1) as cpool,
            tc.tile_pool(name="wpool", bufs=8) as wpool,
            tc.tile_pool(name="dpool", bufs=3) as dpool,
        ):
            # ---- SBUF tiles -------------------------------------------
            # exp bias (-SHIFT) as a per-partition vector
            negs = cpool.tile([PB, 1], F32)
            # x^T (normalized, G-scaled) as [128, KSUB, B]: d = ksub*128 + p
            xsb = cpool.tile([PB, KSUB, B], FP8)
            # warmup operands: real K=128 stationary so the PE ramps
            warm_s = cpool.tile([PB, PB], BF16)
            warm_m = cpool.tile([PB, CHUNK], BF16)
            # per (b-block, group) partial row-sums of exp(logit - SHIFT)
            partials = cpool.tile([PB, BBLK * NGROUPS], F32)

            # memsets live on vector (no DMA-issue capability, otherwise
            # idle); warm tiles first so the warm-up matmuls start ASAP.
            nc.vector.memset(warm_s, 0.0)
            nc.vector.memset(warm_m, 0.0)
            nc.vector.memset(negs, -SHIFT)

            # ---- DMA issue plan ---------------------------------------
            # 15 dma_starts total.  Gen order drives arrival order; all
            # gens complete by ~11.5us, so group g's weights land well
            # before the EXP stream reaches it.
            #   scalar: tail block only (1 gen), then the ACT table load.
            #   sync:   xsb ks0-1, g1 halves, g3 halves, g5 halves.
            #   gpsimd: xsb ks2-3, g2 halves, g4 halves, g6 halves.
            wt_tiles = []
            wt_offs = []
            off = 0
            for col0, ncols in GROUPS:
                nq = math.ceil(ncols / CHUNK)
                cq = min(CHUNK, ncols)
                wt_tiles.append(
                    wpool.tile(
                        [PB, nq, KSUB, cq], FP8, name="wt", tag="w",
                        padded_shape=[PB, GROUP, KSUB, CHUNK],
                    )
                )
                wt_offs.append(off)
                off += nq * KSUB * cq
            assert off == KSUB * CS

            def issue_wt(gi, q0, nq, eng):
                cq = wt_tiles[gi].shape[3]
                o = wt_offs[gi] + q0 * KSUB * cq
                n = nq * KSUB * cq
                eng.dma_start(
                    out=wt_tiles[gi][:, q0 : q0 + nq, :, :],
                    in_=wnt_ext[:, o : o + n].rearrange(
                        "p (q k c) -> p q k c", q=nq, k=KSUB
                    ),
                )

            def issue_xsb(k0, nk, eng):
                o = k0 * B
                eng.dma_start(
                    out=xsb[:, k0 : k0 + nk, :],
                    in_=xnt_ext[:, o : o + nk * B].rearrange(
                        "p (k b) -> p k b", k=nk
                    ),
                )

            # tail block first on sync (its stream starts earliest; scalar
            # keeps only the ACT table load before its first EXP)
            issue_wt(0, 0, 1, nc.sync)
            issue_xsb(0, 2, nc.sync)
            issue_xsb(2, 2, nc.gpsimd)
            # each group split across both engines' ring sets so its two
            # halves flow in parallel
            for gi in range(1, NGROUPS):
                nq = wt_tiles[gi].shape[1]
                issue_wt(gi, 0, nq // 2, nc.sync)
                issue_wt(gi, nq // 2, nq - nq // 2, nc.gpsimd)

            # ---- compute ----------------------------------------------
            with tc.tile_pool(name="psmain", bufs=2, space="PSUM") as pspool:
                # PE warm-up: full-array (K=128) matmuls so the p-state
                # governor ramps to 2.4GHz before the first real matmul.
                warm_ps = pspool.tile(
                    [PB, GROUP, CHUNK], F32, name="warm_ps", tag="ps",
                )
                for _ in range(N_WARM):
                    nc.tensor.matmul(
                        out=warm_ps[:, 0, :], lhsT=warm_s, rhs=warm_m,
                        start=True, stop=True,
                    )

                for gi, (col0, ncols) in enumerate(GROUPS):
                    nsub = math.ceil(ncols / CHUNK)
                    wt = wt_tiles[gi]
                    for bb in range(BBLK):
                        ps = pspool.tile(
                            [PB, nsub, CHUNK], F32, name="ps", tag="ps",
                            padded_shape=[PB, GROUP, CHUNK],
                        )
                        for k2 in range(KSUB // 2):
                            for sub in range(nsub):
                                cn = min(CHUNK, ncols - sub * CHUNK)
                                nc.tensor.matmul(
                                    out=ps[:, sub, :cn],
                                    lhsT=xsb[
                                        :, 2 * k2 : 2 * k2 + 2,
                                        bb * PB : (bb + 1) * PB,
                                    ],
                                    rhs=wt[:, sub, 2 * k2 : 2 * k2 + 2, :cn],
                                    start=(k2 == 0),
                                    stop=(k2 == KSUB // 2 - 1),
                                    perf_mode=mybir.MatmulPerfMode.DoubleRow,
                                )
                        dump = dpool.tile(
                            [PB, nsub, CHUNK], BF16, name="dump", tag="dump",
                            padded_shape=[PB, GROUP, CHUNK],
                        )
                        # exp(EXP_SCALE * cos' - SHIFT), accumulated per row.
                        # ragged tail group (nsub=1, 212 cols) reads exactly
                        # its columns; full groups read nsub*512.
                        if ncols % CHUNK == 0:
                            in_ap = ps[:, :, :]
                            out_ap = dump[:, :, :]
                        else:
                            in_ap = ps[:, 0, :ncols]
                            out_ap = dump[:, 0, :ncols]
                        pcol = partials[
                            :, bb * NGROUPS + gi : bb * NGROUPS + gi + 1
                        ]
                        # ~1/3 of the big tiles hand their row-sum to the
                        # otherwise-idle VectorE (reduce over the bf16 dump)
                        # to shave the 182ns ACTIVATION_READ_ACCUMULATOR off
                        # the ScalarE critical path.
                        # (last group excluded: a trailing DVE reduce would
                        # gate the final output DMA behind the DVE drain)
                        off_dve = gi == 2 or (
                            3 <= gi < NGROUPS - 1
                            and ((gi - 3) * BBLK + bb) % 3 == 0
                        )
                        nc.scalar.activation(
                            out=out_ap,
                            in_=in_ap,
                            func=mybir.ActivationFunctionType.Exp,
                            bias=negs,
                            scale=EXP_SCALE,
                            accum_out=None if off_dve else pcol,
                        )
                        if off_dve:
                            nc.vector.tensor_reduce(
                                pcol,
                                out_ap,
                                axis=mybir.AxisListType.XY,
                                op=mybir.AluOpType.add,
                            )

            # raw partials out (host sums the group columns per row).
            # Groups 0..5's columns ride on idle sync; the final group's 4
            # columns go out on scalar right after its last accumulator
            # read (no cross-engine semaphore on the critical tail).
            pview = partials.rearrange("p (b g) -> p b g", b=BBLK)
            zview = zp_ext.rearrange("p (b g) -> p b g", b=BBLK)
            nc.sync.dma_start(
                out=zview[:, :, 0 : NGROUPS - 1], in_=pview[:, :, 0 : NGROUPS - 1]
            )
            nc.scalar.dma_start(
                out=zview[:, :, NGROUPS - 1 : NGROUPS],
                in_=pview[:, :, NGROUPS - 1 : NGROUPS],
            )

    nc.finalize()
    return nc


def prepare_inputs(x, weight, label, ncores: int = NCORES):
    """Host-side prep: normalize, transpose, G-scale, cast fp8, shard.

    Returns (in_maps, lc2) where lc2[p, j] = SHIFT - S*cos(x_b, w_label_b)
    for b = j*128 + p."""
    x = np.asarray(x, dtype=np.float32)
    weight = np.asarray(weight, dtype=np.float32)
    label = np.asarray(label).astype(np.int64)

    xn = x / np.maximum(
        np.sqrt(np.einsum("bd,bd->b", x, x, dtype=np.float64))[:, None], EPS
    ).astype(np.float32)
    wnorm = np.sqrt(np.einsum("cd,cd->c", weight, weight, dtype=np.float64))
    wn = weight / np.maximum(wnorm[:, None], EPS).astype(np.float32)

    # label cosine computed on host in f64 (exact vs fp32 reference)
    wl = wn[label]  # [B, D]
    label_cos = np.einsum("bd,bd->b", xn.astype(np.float64), wl.astype(np.float64))
    lc2 = (SHIFT - S * label_cos).astype(np.float64)  # [B]
    lc2_pj = np.ascontiguousarray(lc2.reshape(BBLK, PB).T)  # [128, BBLK]

    xnt = np.ascontiguousarray((G * xn).T).astype(NP_FP8)  # [D, B]
    wnt = np.ascontiguousarray((G * wn).T.astype(NP_FP8))  # [D, C]

    # device layouts (see build_nc): per-partition contiguous packing.
    # xnt_p[p, ks*B + b] = xnt[ks*128 + p, b]
    xnt_p = np.ascontiguousarray(
        xnt.reshape(KSUB, PB, B).transpose(1, 0, 2).reshape(PB, KSUB * B)
    )

    in_maps = []
    for i in range(ncores):
        shard = wnt[:, i * CS : (i + 1) * CS]  # [D, CS]
        blocks = []
        for col0, ncols in GROUPS:
            blk = shard[:, col0 : col0 + ncols]  # [D, ncols]
            nq = math.ceil(ncols / CHUNK)
            cq = min(CHUNK, ncols)
            # [ks, p, q, c] -> [p, q, ks, c]
            b4 = blk.reshape(KSUB, PB, nq, cq).transpose(1, 2, 0, 3)
            blocks.append(b4.reshape(PB, nq * KSUB * cq))
        wnt_p = np.ascontiguousarray(np.concatenate(blocks, axis=1))
        in_maps.append({"wnt": wnt_p, "xnt": xnt_p})
    return in_maps, lc2_pj


_NC_CACHE = {}


def _get_nc():
    if "nc" not in _NC_CACHE:
        _NC_CACHE["nc"] = build_nc()
    return _NC_CACHE["nc"]


def _install_ntff_hook():
    """The agent image's antenv lacks axon_hooks; shim it so trace=True can
    capture NTFF profiles via the ctypes hook in trn_agent_boot."""
    import sys
    import types

    try:
        from antenv.axon_hooks import get_axon_ntff_profile_hook  # noqa: F401
        return
    except ImportError:
        pass
    mod = types.ModuleType("antenv.axon_hooks")
    _state = {"hook": None}
    mod.set_axon_ntff_profile_hook = lambda h: _state.__setitem__("hook", h)
    mod.get_axon_ntff_profile_hook = lambda: _state["hook"]
    sys.modules["antenv.axon_hooks"] = mod
    import antenv

    antenv.axon_hooks = mod
    from trn_agent_boot.trn_boot import _ntff_profile_via_ctypes

    mod.set_axon_ntff_profile_hook(
        _ntff_profile_via_ctypes("/opt/axon/libaxon_pjrt.so")
    )
    # keep trace artifacts local (no external upload from this sandbox)
    import concourse.bass_utils as bu

    bu.upload_artifacts = lambda tmpdir: tmpdir


def finish_loss(results, lc2_pj):
    """Host epilogue: sum the 8 cores' per-group partials, log, add label
    term, mean."""
    Z = np.zeros((PB, BBLK), dtype=np.float64)
    for r in results:
        zp = r["zp"].astype(np.float64).reshape(PB, BBLK, NGROUPS)
        Z += zp.sum(axis=2)
    loss = float((np.log(Z) + lc2_pj).mean())
    return np.float32(loss)


def run(x, weight, label, trace=False):
    """Returns (loss_scalar, BassKernelResults)."""
    if trace:
        _install_ntff_hook()
    nc = _get_nc()
    in_maps, lc2_pj = prepare_inputs(x, weight, label)
    res = run_bass_kernel_spmd(
        nc, in_maps, core_ids=list(range(NCORES)), trace=trace
    )
    loss = finish_loss(res.results, lc2_pj)
    return loss, res


def kernel(x, weight, label, batch=None, **_ignored):
    loss, _ = run(x, weight, label, trace=False)
    return np.asarray(loss, dtype=np.float32)


# revision 31
# speedup vs baseline: 1.0259x; 1.0259x over previous
"""ArcFace loss kernel for 8 TRN2 NeuronCores (vocab/tensor-parallel).

reference:
    xn = normalize(x)               # [B, D]
    wn = normalize(weight)          # [C, D]
    logits = 64 * xn @ wn.T         # [B, C]
    loss = mean(CE(logits, label))

Strategy: shard classes C=100000 over 8 cores (12500 each, exact - no
padding). Host prepares normalized, transposed fp8(e4m3) operands scaled
by G=8 (so device cosines are 64*cos and the exp scale is 1); each core
computes its logit shard with TensorE fp8 DoubleRow matmuls (K=256 per
op) into fp32 PSUM and a fused exp+row-sum on ScalarE with a fixed shift
(logsumexp(l) = SHIFT + log(sum(exp(l - SHIFT))), exact since l <= 64).

v2 pipeline notes (from trace analysis of the 77.8us baseline):
  - ScalarE EXP stream is the pacer (~55us busy incl per-op overhead);
    critical path = preamble(5.7us fixed) + time-to-first-EXP + packed
    EXP stream + drain.
  - The 212-class tail chunk is processed as the FIRST group: it needs
    only 106KB of weights, so its 4 small EXPs start ~level with the
    first weight DMAs landing and fill the window while the first full
    1MB group streams in.
  - Warmup matmuls use a K=128 bf16 stationary (not K=1): the TRN2 PE
    p-state governor needs real array utilization to ramp 0.65->2.4GHz,
    and any PE idle gap during the ramp resets it (427ns/MM at MID).
  - DMA descriptor generation costs ~640ns per dma_start on the issuing
    engine; issue is spread over sync/vector/scalar/gpsimd so the first
    group's pieces are all in flight ~2 waves after the preamble.
  - Each core returns raw per-(row, bblock, group) partials [128, 28];
    the host sums group columns, so no on-device reduce/extra sync.
"""

import math
import numpy as np

import concourse.mybir as mybir
import concourse.tile as tile
from concourse import bacc
from concourse.bass_utils import run_bass_kernel_spmd

# Problem constants (hardcoded per harness contract).
B = 512
D = 512
C = 100000
S = 64.0
SHIFT = 20.0  # logsumexp shift; keeps Z ~1e-2 (HW Ln saturates below ~1e-19)
EPS = 1e-12
G = 8.0      # fp8 pre-scale on both operands: device cos' = G^2 * cos
NCORES = 8
CS = C // NCORES        # classes per core = 12500 (exact, no padding)
CHUNK = 512             # matmul moving free dim = one full PSUM bank
GROUP = 4               # psum banks per exp/accumulate group
PB = 128                # partitions
KSUB = D // PB          # 4 contraction subtiles of 128
BBLK = B // PB          # 4 batch blocks
TAIL = CS - 24 * CHUNK  # 212: ragged tail chunk, processed first
N_WARM = 8             # fat PE warm-up matmuls (K=128) to ramp the clock

F32 = mybir.dt.float32
BF16 = mybir.dt.bfloat16
FP8 = mybir.dt.float8e4
NP_FP8 = mybir.dt.np(FP8)
EXP_SCALE = S / (G * G)  # = 1.0

# groups: [tail 212] + two 2-chunk groups (their 512KB lands early and
# fills the EXP stream while the first 1MB group is still in flight) +
# five full 4-chunk groups. (col0, ncols) per group.
GROUPS = [(24 * CHUNK, TAIL), (0, 2 * CHUNK), (2 * CHUNK, 2 * CHUNK)] + [
    (4 * CHUNK + g * GROUP * CHUNK, GROUP * CHUNK) for g in range(5)
]
NGROUPS = len(GROUPS)  # 8


def build_nc(ncores: int = NCORES):
    """Build the SPMD Bass graph."""
    nc = bacc.Bacc(
        "TRN2",
        target_bir_lowering=False,
        debug=False,
        num_devices=ncores,
    )

    # Host-packed layouts: per-partition contiguous so each group loads in
    # 1-2 dma_starts with multi-KB descriptors (descriptor GENERATION on
    # the issuing engine, ~0.62us per dma_start, is the startup
    # bottleneck -- the fewer dma_starts, the earlier everything lands).
    #   wnt: [128p, 848 + 6*8192] -- tail block [ks][212] then per full
    #        group [q(4)][ks(4)][512] (matmul rhs = [p, q, 2ks, 512]).
    #   xnt: [128p, KSUB*B] -- [ks][b].
    wnt_ext = nc.dram_tensor("wnt", [PB, KSUB * CS], FP8, kind="ExternalInput")
    xnt_ext = nc.dram_tensor("xnt", [PB, KSUB * B], FP8, kind="ExternalInput")
    zp_ext = nc.dram_tensor("zp", [PB, BBLK * NGROUPS], F32, kind="ExternalOutput")

    with tile.TileContext(nc) as tc:
        with (
            tc.tile_pool(name="const", bufs=1) as cpool,
            tc.tile_pool(name="wpool", bufs=8) as wpool,
            tc.tile_pool(name="dpool", bufs=3) as dpool,
        ):
            # ---- SBUF tiles -------------------------------------------
            # exp bias (-SHIFT) as a per-partition vector
            negs = cpool.tile([PB, 1], F32)
            # x^T (normalized, G-scaled) as [128, KSUB, B]: d = ksub*128 + p
            xsb = cpool.tile([PB, KSUB, B], FP8)
            # warmup operands: real K=128 stationary so the PE ramps
            warm_s = cpool.tile([PB, PB], BF16)
            warm_m = cpool.tile([PB, CHUNK], BF16)
            # per (b-block, group) partial row-sums of exp(logit - SHIFT)
            partials = cpool.tile([PB, BBLK * NGROUPS], F32)

            # memsets live on vector (no DMA-issue capability, otherwise
            # idle); warm tiles first so the warm-up matmuls start ASAP.
            nc.vector.memset(warm_s, 0.0)
            nc.vector.memset(warm_m, 0.0)
            nc.vector.memset(negs, -SHIFT)

            # ---- DMA issue plan ---------------------------------------
            # 15 dma_starts total.  Gen order drives arrival order; all
            # gens complete by ~11.5us, so group g's weights land well
            # before the EXP stream reaches it.
            #   scalar: tail block only (1 gen), then the ACT table load.
            #   sync:   xsb ks0-1, g1 halves, g3 halves, g5 halves.
            #   gpsimd: xsb ks2-3, g2 halves, g4 halves, g6 halves.
            wt_tiles = []
            wt_offs = []
            off = 0
            for col0, ncols in GROUPS:
                nq = math.ceil(ncols / CHUNK)
                cq = min(CHUNK, ncols)
                wt_tiles.append(
                    wpool.tile(
                        [PB, nq, KSUB, cq], FP8, name="wt", tag="w",
                        padded_shape=[PB, GROUP, KSUB, CHUNK],
                    )
                )
                wt_offs.append(off)
                off += nq * KSUB * cq
            assert off == KSUB * CS

            def issue_wt(gi, q0, nq, eng):
                cq = wt_tiles[gi].shape[3]
                o = wt_offs[gi] + q0 * KSUB * cq
                n = nq * KSUB * cq
                eng.dma_start(
                    out=wt_tiles[gi][:, q0 : q0 + nq, :, :],
                    in_=wnt_ext[:, o : o + n].rearrange(
                        "p (q k c) -> p q k c", q=nq, k=KSUB
                    ),
                )

            def issue_xsb(k0, nk, eng):
                o = k0 * B
                eng.dma_start(
                    out=xsb[:, k0 : k0 + nk, :],
                    in_=xnt_ext[:, o : o + nk * B].rearrange(
                        "p (k b) -> p k b", k=nk
                    ),
                )

            issue_wt(0, 0, 1, nc.scalar)        # tail block (848B lines)
            issue_xsb(0, 2, nc.sync)
            issue_xsb(2, 2, nc.gpsimd)
            # each group split across both engines' ring sets so its two
            # halves flow in parallel
            for gi in range(1, NGROUPS):
                nq = wt_tiles[gi].shape[1]
                issue_wt(gi, 0, nq // 2, nc.sync)
                issue_wt(gi, nq // 2, nq - nq // 2, nc.gpsimd)

            # ---- compute ----------------------------------------------
            with tc.tile_pool(name="psmain", bufs=2, space="PSUM") as pspool:
                # PE warm-up: full-array (K=128) matmuls so the p-state
                # governor ramps to 2.4GHz before the first real matmul.
                warm_ps = pspool.tile(
                    [PB, GROUP, CHUNK], F32, name="warm_ps", tag="ps",
                )
                for _ in range(N_WARM):
                    nc.tensor.matmul(
                        out=warm_ps[:, 0, :], lhsT=warm_s, rhs=warm_m,
                        start=True, stop=True,
                    )

                for gi, (col0, ncols) in enumerate(GROUPS):
                    nsub = math.ceil(ncols / CHUNK)
                    wt = wt_tiles[gi]
                    for bb in range(BBLK):
                        ps = pspool.tile(
                            [PB, nsub, CHUNK], F32, name="ps", tag="ps",
                            padded_shape=[PB, GROUP, CHUNK],
                        )
                        for k2 in range(KSUB // 2):
                            for sub in range(nsub):
                                cn = min(CHUNK, ncols - sub * CHUNK)
                                nc.tensor.matmul(
                                    out=ps[:, sub, :cn],
                                    lhsT=xsb[
                                        :, 2 * k2 : 2 * k2 + 2,
                                        bb * PB : (bb + 1) * PB,
                                    ],
                                    rhs=wt[:, sub, 2 * k2 : 2 * k2 + 2, :cn],
                                    start=(k2 == 0),
                                    stop=(k2 == KSUB // 2 - 1),
                                    perf_mode=mybir.MatmulPerfMode.DoubleRow,
                                )
                        dump = dpool.tile(
                            [PB, nsub, CHUNK], BF16, name="dump", tag="dump",
                            padded_shape=[PB, GROUP, CHUNK],
                        )
                        # exp(EXP_SCALE * cos' - SHIFT), accumulated per row.
                        # ragged tail group (nsub=1, 212 cols) reads exactly
                        # its columns; full groups read nsub*512.
                        if ncols % CHUNK == 0:
                            in_ap = ps[:, :, :]
                            out_ap = dump[:, :, :]
                        else:
                            in_ap = ps[:, 0, :ncols]
                            out_ap = dump[:, 0, :ncols]
                        pcol = partials[
                            :, bb * NGROUPS + gi : bb * NGROUPS + gi + 1
                        ]
                        # ~1/3 of the big tiles hand their row-sum to the
                        # otherwise-idle VectorE (reduce over the bf16 dump)
                        # to shave the 182ns ACTIVATION_READ_ACCUMULATOR off
                        # the ScalarE critical path.
                        # (last group excluded: a trailing DVE reduce would
                        # gate the final output DMA behind the DVE drain)
                        off_dve = gi == 2 or (
                            3 <= gi < NGROUPS - 1
                            and ((gi - 3) * BBLK + bb) % 3 == 0
                        )
                        nc.scalar.activation(
                            out=out_ap,
                            in_=in_ap,
                            func=mybir.ActivationFunctionType.Exp,
                            bias=negs,
                            scale=EXP_SCALE,
                            accum_out=None if off_dve else pcol,
                        )
                        if off_dve:
                            nc.vector.tensor_reduce(
                                pcol,
                                out_ap,
                                axis=mybir.AxisListType.XY,
                                op=mybir.AluOpType.add,
                            )

            # raw partials out (host sums the group columns per row).
            # Groups 0..5's columns ride on idle sync; the final group's 4
            # columns go out on scalar right after its last accumulator
            # read (no cross-engine semaphore on the critical tail).
            pview = partials.rearrange("p (b g) -> p b g", b=BBLK)
            zview = zp_ext.rearrange("p (b g) -> p b g", b=BBLK)
            nc.sync.dma_start(
                out=zview[:, :, 0 : NGROUPS - 1], in_=pview[:, :, 0 : NGROUPS - 1]
            )
            nc.scalar.dma_start(
                out=zview[:, :, NGROUPS - 1 : NGROUPS],
                in_=pview[:, :, NGROUPS - 1 : NGROUPS],
            )

    nc.finalize()
    return nc


def prepare_inputs(x, weight, label, ncores: int = NCORES):
    """Host-side prep: normalize, transpose, G-scale, cast fp8, shard.

    Returns (in_maps, lc2) where lc2[p, j] = SHIFT - S*cos(x_b, w_label_b)
    for b = j*128 + p."""
    x = np.asarray(x, dtype=np.float32)
    weight = np.asarray(weight, dtype=np.float32)
    label = np.asarray(label).astype(np.int64)

    xn = x / np.maximum(
        np.sqrt(np.einsum("bd,bd->b", x, x, dtype=np.float64))[:, None], EPS
    ).astype(np.float32)
    wnorm = np.sqrt(np.einsum("cd,cd->c", weight, weight, dtype=np.float64))
    wn = weight / np.maximum(wnorm[:, None], EPS).astype(np.float32)

    # label cosine computed on host in f64 (exact vs fp32 reference)
    wl = wn[label]  # [B, D]
    label_cos = np.einsum("bd,bd->b", xn.astype(np.float64), wl.astype(np.float64))
    lc2 = (SHIFT - S * label_cos).astype(np.float64)  # [B]
    lc2_pj = np.ascontiguousarray(lc2.reshape(BBLK, PB).T)  # [128, BBLK]

    xnt = np.ascontiguousarray((G * xn).T).astype(NP_FP8)  # [D, B]
    wnt = np.ascontiguousarray((G * wn).T.astype(NP_FP8))  # [D, C]

    # device layouts (see build_nc): per-partition contiguous packing.
    # xnt_p[p, ks*B + b] = xnt[ks*128 + p, b]
    xnt_p = np.ascontiguousarray(
        xnt.reshape(KSUB, PB, B).transpose(1, 0, 2).reshape(PB, KSUB * B)
    )

    in_maps = []
    for i in range(ncores):
        shard = wnt[:, i * CS : (i + 1) * CS]  # [D, CS]
        blocks = []
        for col0, ncols in GROUPS:
            blk = shard[:, col0 : col0 + ncols]  # [D, ncols]
            nq = math.ceil(ncols / CHUNK)
            cq = min(CHUNK, ncols)
            # [ks, p, q, c] -> [p, q, ks, c]
            b4 = blk.reshape(KSUB, PB, nq, cq).transpose(1, 2, 0, 3)
            blocks.append(b4.reshape(PB, nq * KSUB * cq))
        wnt_p = np.ascontiguousarray(np.concatenate(blocks, axis=1))
        in_maps.append({"wnt": wnt_p, "xnt": xnt_p})
    return in_maps, lc2_pj


_NC_CACHE = {}


def _get_nc():
    if "nc" not in _NC_CACHE:
        _NC_CACHE["nc"] = build_nc()
    return _NC_CACHE["nc"]


def _install_ntff_hook():
    """The agent image's antenv lacks axon_hooks; shim it so trace=True can
    capture NTFF profiles via the ctypes hook in trn_agent_boot."""
    import sys
    import types

    try:
        from antenv.axon_hooks import get_axon_ntff_profile_hook  # noqa: F401
        return
    except ImportError:
        pass
    mod = types.ModuleType("antenv.axon_hooks")
    _state = {"hook": None}
    mod.set_axon_ntff_profile_hook = lambda h: _state.__setitem__("hook", h)
    mod.get_axon_ntff_profile_hook = lambda: _state["hook"]
    sys.modules["antenv.axon_hooks"] = mod
    import antenv

    antenv.axon_hooks = mod
    from trn_agent_boot.trn_boot import _ntff_profile_via_ctypes

    mod.set_axon_ntff_profile_hook(
        _ntff_profile_via_ctypes("/opt/axon/libaxon_pjrt.so")
    )
    # keep trace artifacts local (no external upload from this sandbox)
    import concourse.bass_utils as bu

    bu.upload_artifacts = lambda tmpdir: tmpdir


def finish_loss(results, lc2_pj):
    """Host epilogue: sum the 8 cores' per-group partials, log, add label
    term, mean."""
    Z = np.zeros((PB, BBLK), dtype=np.float64)
    for r in results:
        zp = r["zp"].astype(np.float64).reshape(PB, BBLK, NGROUPS)
        Z += zp.sum(axis=2)
    loss = float((np.log(Z) + lc2_pj).mean())
    return np.float32(loss)


def run(x, weight, label, trace=False):
    """Returns (loss_scalar, BassKernelResults)."""
    if trace:
        _install_ntff_hook()
    nc = _get_nc()
    in_maps, lc2_pj = prepare_inputs(x, weight, label)
    res = run_bass_kernel_spmd(
        nc, in_maps, core_ids=list(range(NCORES)), trace=trace
    )
    loss = finish_loss(res.results, lc2_pj)
    return loss, res


def kernel(x, weight, label, batch=None, **_ignored):
    loss, _ = run(x, weight, label, trace=False)
    return np.asarray(loss, dtype=np.float32)


# revision 41
# speedup vs baseline: 1.0780x; 1.0508x over previous
"""ArcFace loss kernel for 8 TRN2 NeuronCores (vocab/tensor-parallel).

reference:
    xn = normalize(x)               # [B, D]
    wn = normalize(weight)          # [C, D]
    logits = 64 * xn @ wn.T         # [B, C]
    loss = mean(CE(logits, label))

Strategy: shard classes C=100000 over 8 cores (12500 each, exact - no
padding). Host prepares normalized, transposed fp8(e4m3) operands scaled
by G=8 (so device cosines are 64*cos and the exp scale is 1); each core
computes its logit shard with TensorE fp8 DoubleRow matmuls (K=256 per
op) into fp32 PSUM and a fused exp+row-sum on ScalarE with a fixed shift
(logsumexp(l) = SHIFT + log(sum(exp(l - SHIFT))), exact since l <= 64).

v2 pipeline notes (from trace analysis of the 77.8us baseline):
  - ScalarE EXP stream is the pacer (~55us busy incl per-op overhead);
    critical path = preamble(5.7us fixed) + time-to-first-EXP + packed
    EXP stream + drain.
  - The 212-class tail chunk is processed as the FIRST group: it needs
    only 106KB of weights, so its 4 small EXPs start ~level with the
    first weight DMAs landing and fill the window while the first full
    1MB group streams in.
  - Warmup matmuls use a K=128 bf16 stationary (not K=1): the TRN2 PE
    p-state governor needs real array utilization to ramp 0.65->2.4GHz,
    and any PE idle gap during the ramp resets it (427ns/MM at MID).
  - DMA descriptor generation costs ~640ns per dma_start on the issuing
    engine; issue is spread over sync/vector/scalar/gpsimd so the first
    group's pieces are all in flight ~2 waves after the preamble.
  - Each core returns raw per-(row, bblock, group) partials [128, 28];
    the host sums group columns, so no on-device reduce/extra sync.
"""

import math
import numpy as np

import concourse.mybir as mybir
import concourse.tile as tile
from concourse import bacc
from concourse.bass_utils import run_bass_kernel_spmd

# Problem constants (hardcoded per harness contract).
B = 512
D = 512
C = 100000
S = 64.0
SHIFT = 20.0  # logsumexp shift; keeps Z ~1e-2 (HW Ln saturates below ~1e-19)
EPS = 1e-12
G = 8.0      # fp8 pre-scale on both operands: device cos' = G^2 * cos
NCORES = 8
CS = C // NCORES        # classes per core = 12500 (exact, no padding)
CHUNK = 512             # matmul moving free dim = one full PSUM bank
GROUP = 4               # psum banks per exp/accumulate group
PB = 128                # partitions
KSUB = D // PB          # 4 contraction subtiles of 128
BBLK = B // PB          # 4 batch blocks
TAIL = CS - 24 * CHUNK  # 212: ragged tail chunk, processed first
N_WARM = 8             # fat PE warm-up matmuls (K=128) to ramp the clock

F32 = mybir.dt.float32
BF16 = mybir.dt.bfloat16
FP8 = mybir.dt.float8e4
NP_FP8 = mybir.dt.np(FP8)
EXP_SCALE = S / (G * G)  # = 1.0

# groups: [tail 212] + two 2-chunk groups (their 512KB lands early and
# fills the EXP stream while the first 1MB group is still in flight) +
# five full 4-chunk groups. (col0, ncols) per group.
GROUPS = [(24 * CHUNK, TAIL), (0, 2 * CHUNK), (2 * CHUNK, 2 * CHUNK)] + [
    (4 * CHUNK + g * GROUP * CHUNK, GROUP * CHUNK) for g in range(5)
]
NGROUPS = len(GROUPS)  # 8


def build_nc(ncores: int = NCORES):
    """Build the SPMD Bass graph."""
    nc = bacc.Bacc(
        "TRN2",
        target_bir_lowering=False,
        debug=False,
        num_devices=ncores,
    )

    # Host-packed layouts: per-partition contiguous so each group loads in
    # 1-2 dma_starts with multi-KB descriptors (descriptor GENERATION on
    # the issuing engine, ~0.62us per dma_start, is the startup
    # bottleneck -- the fewer dma_starts, the earlier everything lands).
    #   wnt: [128p, 848 + 6*8192] -- tail block [ks][212] then per full
    #        group [q(4)][ks(4)][512] (matmul rhs = [p, q, 2ks, 512]).
    #   xnt: [128p, KSUB*B] -- [ks][b].
    wnt_ext = nc.dram_tensor("wnt", [PB, KSUB * CS], FP8, kind="ExternalInput")
    xnt_ext = nc.dram_tensor("xnt", [PB, KSUB * B], FP8, kind="ExternalInput")
    # [0:32) = per-(bblock, group) partials; [32:48) = A/B pair-EXP
    # per-bank partial sums (see finish_loss for the unpacking)
    zp_ext = nc.dram_tensor(
        "zp", [PB, BBLK * NGROUPS + 16], F32, kind="ExternalOutput"
    )

    with tile.TileContext(nc) as tc:
        with (
            tc.tile_pool(name="const", bufs=1) as cpool,
            tc.tile_pool(name="wpool", bufs=8) as wpool,
            tc.tile_pool(name="dpool", bufs=3) as dpool,
        ):
            # ---- SBUF tiles -------------------------------------------
            # exp bias (-SHIFT) as a per-partition vector
            negs = cpool.tile([PB, 1], F32)
            # x^T (normalized, G-scaled) as [128, KSUB, B]: d = ksub*128 + p
            xsb = cpool.tile([PB, KSUB, B], FP8)
            # warmup operands: real K=128 stationary so the PE ramps
            warm_s = cpool.tile([PB, PB], BF16)
            warm_m = cpool.tile([PB, CHUNK], BF16)
            # per (b-block, group) partial row-sums of exp(logit - SHIFT)
            partials = cpool.tile([PB, BBLK * NGROUPS], F32)
            # A/B pair-EXP per-bank sums: [pair(4), bank(4)]
            partials_ab = cpool.tile([PB, 16], F32)

            # memsets live on vector (no DMA-issue capability, otherwise
            # idle); warm tiles first so the warm-up matmuls start ASAP.
            nc.vector.memset(warm_s, 0.0)
            nc.vector.memset(warm_m, 0.0)
            nc.vector.memset(negs, -SHIFT)

            # ---- DMA issue plan ---------------------------------------
            # 15 dma_starts total.  Gen order drives arrival order; all
            # gens complete by ~11.5us, so group g's weights land well
            # before the EXP stream reaches it.
            #   scalar: tail block only (1 gen), then the ACT table load.
            #   sync:   xsb ks0-1, g1 halves, g3 halves, g5 halves.
            #   gpsimd: xsb ks2-3, g2 halves, g4 halves, g6 halves.
            wt_tiles = []
            wt_offs = []
            off = 0
            for col0, ncols in GROUPS:
                nq = math.ceil(ncols / CHUNK)
                cq = min(CHUNK, ncols)
                wt_tiles.append(
                    wpool.tile(
                        [PB, nq, KSUB, cq], FP8, name="wt", tag="w",
                        padded_shape=[PB, GROUP, KSUB, CHUNK],
                    )
                )
                wt_offs.append(off)
                off += nq * KSUB * cq
            assert off == KSUB * CS

            def issue_wt(gi, q0, nq, eng):
                cq = wt_tiles[gi].shape[3]
                o = wt_offs[gi] + q0 * KSUB * cq
                n = nq * KSUB * cq
                eng.dma_start(
                    out=wt_tiles[gi][:, q0 : q0 + nq, :, :],
                    in_=wnt_ext[:, o : o + n].rearrange(
                        "p (q k c) -> p q k c", q=nq, k=KSUB
                    ),
                )

            def issue_xsb(k0, nk, eng):
                o = k0 * B
                eng.dma_start(
                    out=xsb[:, k0 : k0 + nk, :],
                    in_=xnt_ext[:, o : o + nk * B].rearrange(
                        "p (k b) -> p k b", k=nk
                    ),
                )

            issue_wt(0, 0, 1, nc.scalar)        # tail block (848B lines)
            issue_xsb(0, 2, nc.sync)
            issue_xsb(2, 2, nc.gpsimd)
            # each group split across both engines' ring sets so its two
            # halves flow in parallel
            for gi in range(1, NGROUPS):
                nq = wt_tiles[gi].shape[1]
                issue_wt(gi, 0, nq // 2, nc.sync)
                issue_wt(gi, nq // 2, nq - nq // 2, nc.gpsimd)

            # ---- compute ----------------------------------------------
            def do_group(pool, gi, pad_banks):
                col0, ncols = GROUPS[gi]
                nsub = math.ceil(ncols / CHUNK)
                wt = wt_tiles[gi]
                for bb in range(BBLK):
                    ps = pool.tile(
                        [PB, nsub, CHUNK], F32, name="ps", tag="ps",
                        padded_shape=[PB, pad_banks, CHUNK],
                    )
                    for k2 in range(KSUB // 2):
                        for sub in range(nsub):
                            cn = min(CHUNK, ncols - sub * CHUNK)
                            nc.tensor.matmul(
                                out=ps[:, sub, :cn],
                                lhsT=xsb[
                                    :, 2 * k2 : 2 * k2 + 2,
                                    bb * PB : (bb + 1) * PB,
                                ],
                                rhs=wt[:, sub, 2 * k2 : 2 * k2 + 2, :cn],
                                start=(k2 == 0),
                                stop=(k2 == KSUB // 2 - 1),
                                perf_mode=mybir.MatmulPerfMode.DoubleRow,
                            )
                    dump = dpool.tile(
                        [PB, nsub, CHUNK], BF16, name="dump", tag="dump",
                        padded_shape=[PB, GROUP, CHUNK],
                    )
                    # exp(EXP_SCALE * cos' - SHIFT), accumulated per row.
                    # ragged tail group (nsub=1, 212 cols) reads exactly
                    # its columns; full groups read nsub*512.
                    if ncols % CHUNK == 0:
                        in_ap = ps[:, :, :]
                        out_ap = dump[:, :, :]
                    else:
                        in_ap = ps[:, 0, :ncols]
                        out_ap = dump[:, 0, :ncols]
                    pcol = partials[
                        :, bb * NGROUPS + gi : bb * NGROUPS + gi + 1
                    ]
                    # ~1/3 of the big tiles hand their row-sum to the
                    # otherwise-idle VectorE (reduce over the bf16 dump)
                    # to shave the 182ns ACTIVATION_READ_ACCUMULATOR off
                    # the ScalarE critical path.
                    # (last group excluded: a trailing DVE reduce would
                    # gate the final output DMA behind the DVE drain)
                    off_dve = gi == 2 or (
                        3 <= gi < NGROUPS - 1
                        and ((gi - 3) * BBLK + bb) % 3 == 0
                    )
                    nc.scalar.activation(
                        out=out_ap,
                        in_=in_ap,
                        func=mybir.ActivationFunctionType.Exp,
                        bias=negs,
                        scale=EXP_SCALE,
                        accum_out=None if off_dve else pcol,
                    )
                    if off_dve:
                        nc.vector.tensor_reduce(
                            pcol,
                            out_ap,
                            axis=mybir.AxisListType.XY,
                            op=mybir.AluOpType.add,
                        )

            with tc.tile_pool(name="psmain", bufs=2, space="PSUM") as pspool:
                # PE warm-up: full-array (K=128) matmuls so the p-state
                # governor ramps to 2.4GHz before the first real matmul.
                warm_ps = pspool.tile(
                    [PB, GROUP, CHUNK], F32, name="warm_ps", tag="ps",
                )
                for _ in range(N_WARM):
                    nc.tensor.matmul(
                        out=warm_ps[:, 0, :], lhsT=warm_s, rhs=warm_m,
                        start=True, stop=True,
                    )

                # Tail group: one 4-bank tile, bank per batch block -- a
                # single N=848 EXP (no accum) replaces 4 EXP+RA pairs; the
                # per-block row-sums come from one DVE reduce over the
                # bf16 dump (axis=X keeps the bank dim).
                ps_t = pspool.tile(
                    [PB, GROUP, CHUNK], F32, name="ps", tag="ps",
                )
                wt0 = wt_tiles[0]
                for bb in range(BBLK):
                    for k2 in range(KSUB // 2):
                        nc.tensor.matmul(
                            out=ps_t[:, bb, :TAIL],
                            lhsT=xsb[
                                :, 2 * k2 : 2 * k2 + 2,
                                bb * PB : (bb + 1) * PB,
                            ],
                            rhs=wt0[:, 0, 2 * k2 : 2 * k2 + 2, :TAIL],
                            start=(k2 == 0),
                            stop=(k2 == KSUB // 2 - 1),
                            perf_mode=mybir.MatmulPerfMode.DoubleRow,
                        )
                dump_t = dpool.tile(
                    [PB, GROUP, CHUNK], BF16, name="dump", tag="dump",
                )
                nc.scalar.activation(
                    out=dump_t[:, :, :TAIL],
                    in_=ps_t[:, :, :TAIL],
                    func=mybir.ActivationFunctionType.Exp,
                    bias=negs,
                    scale=EXP_SCALE,
                )
                nc.vector.tensor_reduce(
                    partials.rearrange("p (b g) -> p b g", b=BBLK)[:, :, 0],
                    dump_t[:, :, :TAIL],
                    axis=mybir.AxisListType.X,
                    op=mybir.AluOpType.add,
                )

                # A/B groups (2 chunks each): batch-block PAIRS share one
                # 4-bank tile (bank = j*2+sub) -> one N=2048 EXP per pair
                # instead of two N=1024 EXP+RA pairs; per-bank sums via
                # DVE, host adds the bank pairs.
                for gi in (1, 2):
                    col0, ncols = GROUPS[gi]
                    wt = wt_tiles[gi]
                    for j2 in range(2):
                        ps = pspool.tile(
                            [PB, GROUP, CHUNK], F32, name="ps", tag="ps",
                        )
                        for j in range(2):
                            bb = 2 * j2 + j
                            for k2 in range(KSUB // 2):
                                for sub in range(2):
                                    nc.tensor.matmul(
                                        out=ps[:, 2 * j + sub, :],
                                        lhsT=xsb[
                                            :, 2 * k2 : 2 * k2 + 2,
                                            bb * PB : (bb + 1) * PB,
                                        ],
                                        rhs=wt[
                                            :, sub, 2 * k2 : 2 * k2 + 2, :
                                        ],
                                        start=(k2 == 0),
                                        stop=(k2 == KSUB // 2 - 1),
                                        perf_mode=mybir.MatmulPerfMode.DoubleRow,
                                    )
                        dump = dpool.tile(
                            [PB, GROUP, CHUNK], BF16, name="dump", tag="dump",
                        )
                        nc.scalar.activation(
                            out=dump,
                            in_=ps,
                            func=mybir.ActivationFunctionType.Exp,
                            bias=negs,
                            scale=EXP_SCALE,
                        )
                        pk = (gi - 1) * 8 + j2 * 4
                        nc.vector.tensor_reduce(
                            partials_ab[:, pk : pk + 4],
                            dump,
                            axis=mybir.AxisListType.X,
                            op=mybir.AluOpType.add,
                        )

                for gi in range(3, NGROUPS):
                    do_group(pspool, gi, GROUP)

            # raw partials out (host sums the group columns per row).
            # Earlier groups' columns ride on idle sync; the final group's
            # 4 columns go out on scalar right after its last accumulator
            # read (no cross-engine semaphore on the critical tail).
            pview = partials.rearrange("p (b g) -> p b g", b=BBLK)
            zmain = zp_ext[:, 0 : BBLK * NGROUPS].rearrange(
                "p (b g) -> p b g", b=BBLK
            )
            nc.sync.dma_start(
                out=zmain[:, :, 0 : NGROUPS - 1], in_=pview[:, :, 0 : NGROUPS - 1]
            )
            nc.sync.dma_start(
                out=zp_ext[:, BBLK * NGROUPS :], in_=partials_ab
            )
            nc.scalar.dma_start(
                out=zmain[:, :, NGROUPS - 1 : NGROUPS],
                in_=pview[:, :, NGROUPS - 1 : NGROUPS],
            )

    nc.finalize()
    return nc


def prepare_inputs(x, weight, label, ncores: int = NCORES):
    """Host-side prep: normalize, transpose, G-scale, cast fp8, shard.

    Returns (in_maps, lc2) where lc2[p, j] = SHIFT - S*cos(x_b, w_label_b)
    for b = j*128 + p."""
    x = np.asarray(x, dtype=np.float32)
    weight = np.asarray(weight, dtype=np.float32)
    label = np.asarray(label).astype(np.int64)

    xn = x / np.maximum(
        np.sqrt(np.einsum("bd,bd->b", x, x, dtype=np.float64))[:, None], EPS
    ).astype(np.float32)
    wnorm = np.sqrt(np.einsum("cd,cd->c", weight, weight, dtype=np.float64))
    wn = weight / np.maximum(wnorm[:, None], EPS).astype(np.float32)

    # label cosine computed on host in f64 (exact vs fp32 reference)
    wl = wn[label]  # [B, D]
    label_cos = np.einsum("bd,bd->b", xn.astype(np.float64), wl.astype(np.float64))
    lc2 = (SHIFT - S * label_cos).astype(np.float64)  # [B]
    lc2_pj = np.ascontiguousarray(lc2.reshape(BBLK, PB).T)  # [128, BBLK]

    xnt = np.ascontiguousarray((G * xn).T).astype(NP_FP8)  # [D, B]
    wnt = np.ascontiguousarray((G * wn).T.astype(NP_FP8))  # [D, C]

    # device layouts (see build_nc): per-partition contiguous packing.
    # xnt_p[p, ks*B + b] = xnt[ks*128 + p, b]
    xnt_p = np.ascontiguousarray(
        xnt.reshape(KSUB, PB, B).transpose(1, 0, 2).reshape(PB, KSUB * B)
    )

    in_maps = []
    for i in range(ncores):
        shard = wnt[:, i * CS : (i + 1) * CS]  # [D, CS]
        blocks = []
        for col0, ncols in GROUPS:
            blk = shard[:, col0 : col0 + ncols]  # [D, ncols]
            nq = math.ceil(ncols / CHUNK)
            cq = min(CHUNK, ncols)
            # [ks, p, q, c] -> [p, q, ks, c]
            b4 = blk.reshape(KSUB, PB, nq, cq).transpose(1, 2, 0, 3)
            blocks.append(b4.reshape(PB, nq * KSUB * cq))
        wnt_p = np.ascontiguousarray(np.concatenate(blocks, axis=1))
        in_maps.append({"wnt": wnt_p, "xnt": xnt_p})
    return in_maps, lc2_pj


_NC_CACHE = {}


def _get_nc():
    if "nc" not in _NC_CACHE:
        _NC_CACHE["nc"] = build_nc()
    return _NC_CACHE["nc"]


def _install_ntff_hook():
    """The agent image's antenv lacks axon_hooks; shim it so trace=True can
    capture NTFF profiles via the ctypes hook in trn_agent_boot."""
    import sys
    import types

    try:
        from antenv.axon_hooks import get_axon_ntff_profile_hook  # noqa: F401
        return
    except ImportError:
        pass
    mod = types.ModuleType("antenv.axon_hooks")
    _state = {"hook": None}
    mod.set_axon_ntff_profile_hook = lambda h: _state.__setitem__("hook", h)
    mod.get_axon_ntff_profile_hook = lambda: _state["hook"]
    sys.modules["antenv.axon_hooks"] = mod
    import antenv

    antenv.axon_hooks = mod
    from trn_agent_boot.trn_boot import _ntff_profile_via_ctypes

    mod.set_axon_ntff_profile_hook(
        _ntff_profile_via_ctypes("/opt/axon/libaxon_pjrt.so")
    )
    # keep trace artifacts local (no external upload from this sandbox)
    import concourse.bass_utils as bu

    bu.upload_artifacts = lambda tmpdir: tmpdir


def finish_loss(results, lc2_pj):
    """Host epilogue: sum the 8 cores' per-group partials, log, add label
    term, mean.  Columns for groups 1-2 live in the pair-EXP per-bank
    block [32:48) = [pair(4), bank(4)]: pair k covers batch blocks
    (k%2)*2 + {0,1}, bank m belongs to block offset m//2."""
    Z = np.zeros((PB, BBLK), dtype=np.float64)
    for r in results:
        zp = r["zp"].astype(np.float64)
        main = zp[:, : BBLK * NGROUPS].reshape(PB, BBLK, NGROUPS)
        Z += main[:, :, 0]
        Z += main[:, :, 3:].sum(axis=2)
        ab = zp[:, BBLK * NGROUPS :].reshape(PB, 4, 2, 2)
        for k in range(4):
            for j in range(2):
                Z[:, (k % 2) * 2 + j] += ab[:, k, j, :].sum(axis=1)
    loss = float((np.log(Z) + lc2_pj).mean())
    return np.float32(loss)


def run(x, weight, label, trace=False):
    """Returns (loss_scalar, BassKernelResults)."""
    if trace:
        _install_ntff_hook()
    nc = _get_nc()
    in_maps, lc2_pj = prepare_inputs(x, weight, label)
    res = run_bass_kernel_spmd(
        nc, in_maps, core_ids=list(range(NCORES)), trace=trace
    )
    loss = finish_loss(res.results, lc2_pj)
    return loss, res


def kernel(x, weight, label, batch=None, **_ignored):
    loss, _ = run(x, weight, label, trace=False)
    return np.asarray(loss, dtype=np.float32)
